# revision 9
# baseline (speedup 1.0000x reference)
"""Single-head causal attention with RoPE on 8 TRN2 NeuronCores (v5).

Sharding: core c -> batch c//2, parity p = c%2 owns the interleaved
512-row q-blocks {p, p+2, p+4, p+6} of T=4096. Each core projects
q/k/v + RoPE only for its OWN 2048 rows; pairs exchange post-RoPE kT
and V via chunked fp16 AllGathers into a parity-ordered kT layout
(parity-0 blocks = kT slots 0-3, parity-1 = slots 4-7), identical
SPMD program on all cores.

v5 changes vs v4:
- q/k projections computed directly in [d, t] layout (weights
  stationary, x streamed) -> RoPE runs on [d, t] tiles with
  partition-offset DVE ops; NO PE transposes or ACT copies for q/k.
- v still projected [d, t] then PE-transposed (4 tiles/block) into
  [s, d] for the exchange.
- Head: DMA emission in exact consumption order, fine-grained first
  chunks, causal masks generated on-device (affine_select + parity
  input) instead of 2MB host DMA.
- Tail: denominator partials merged on-device (2 DVE adds per q-slot)
  -> accout shrinks 2MB -> 0.5MB.
- cin/ingest DMAs ride the scalar HWDGE queue; outputs ride sync, so
  collective-dependent descriptors never block output descriptors.
"""
import numpy as np

B, T, C, HD = 4, 4096, 2048, 128
P = 128
BS = 512
SCALE = float(C) ** -0.5


def build():
    import concourse.bass as bass
    import concourse.mybir as mybir
    import bass_rust
    from concourse.tile import TileContext
    from concourse.masks import make_identity

    f32 = mybir.dt.float32
    f16 = mybir.dt.float16
    EXP = mybir.ActivationFunctionType.Exp

    nc = bass.Bass(num_devices=8)
    # host layouts (see _prep_inputs):
    # xt[p, tg, ci, t] flat; w2[p, (proj k|v|q, ci, d)] flat;
    # csb/snb[d-row, (tb, t)]; par = parity scalar per partition
    xt = nc.declare_dram_parameter("xt", [P, 4 * 16 * BS], f16, isOutput=False)
    w2 = nc.declare_dram_parameter("w2", [P, 3 * 16 * P], f16, isOutput=False)
    csb = nc.declare_dram_parameter("csb", [P, 4 * BS], f16, isOutput=False)
    snb = nc.declare_dram_parameter("snb", [P, 4 * BS], f16, isOutput=False)
    par = nc.declare_dram_parameter("par", [P, 1], f32, isOutput=False)
    # unnormalized AV output [d, j*512+q]; merged denominator partials
    # [s_pos, j*512+q] (host sums the 128 partitions and divides)
    o2out = nc.declare_dram_parameter("o2out", [P, 4 * BS], f16, isOutput=True)
    accout = nc.declare_dram_parameter("accout", [P, 4 * BS], f16,
                                       isOutput=True)

    cins = [nc.dram_tensor(f"cin{t}", [P, 2 * BS], f16, kind="Internal")
            for t in range(4)]
    couts = [nc.dram_tensor(f"cout{t}", [2 * P, 2 * BS], f16, kind="Internal")
             for t in range(4)]

    KO, VO, QO = 0, BS, 2 * BS   # pp column offsets (bank-aligned)

    with TileContext(nc) as tc:
        with (
            tc.tile_pool(name="const", bufs=1) as cp,
            tc.tile_pool(name="xp", bufs=1) as xp,
            tc.tile_pool(name="rot", bufs=3) as rp,
            tc.tile_pool(name="vt", bufs=2) as vtp,
            tc.tile_pool(name="pt", bufs=4) as ptp,
        ):
            # ---- input DMAs interleaved in consumption order (sync) ----
            # w2 layout is ci-major: chunk (ci, proj) at col (ci*3+proj)*128
            wt = cp.tile([P, 3 * 16 * P], f16, tag="wt")
            xbig = [None] * 4

            def load_x(tg, interleave_w=False):
                xb = xp.tile([P, 16 * BS], f16, tag=f"xbig{tg}")
                for d in range(4):
                    if interleave_w:
                        ws = slice(d * 12 * P, (d + 1) * 12 * P)
                        nc.sync.dma_start(wt[:, ws], w2[:, ws])
                    base = tg * 16 * BS + d * 4 * BS
                    nc.sync.dma_start(
                        xb[:, d * 4 * BS:(d + 1) * 4 * BS],
                        xt[:, base:base + 4 * BS])
                xbig[tg] = xb

            load_x(0, interleave_w=True)
            cst = cp.tile([P, 4 * BS], f16, tag="cst")
            snt = cp.tile([P, 4 * BS], f16, tag="snt")
            nc.sync.dma_start(cst[:, 0:BS], csb[:, 0:BS])
            nc.sync.dma_start(snt[:, 0:BS], snb[:, 0:BS])
            parT = cp.tile([P, 1], f32, tag="parT")
            nc.sync.dma_start(parT[:], par[:])
            load_x(1)
            nc.sync.dma_start(cst[:, BS:4 * BS], csb[:, BS:4 * BS])
            nc.sync.dma_start(snt[:, BS:4 * BS], snb[:, BS:4 * BS])
            load_x(2)
            load_x(3)

            ident = cp.tile([P, P], f16, tag="ident")
            make_identity(nc, ident[:])

            # on-device causal masks: tri[p, st*512+q] = (p + 128*st <= q)
            tri = cp.tile([P, 4 * BS], f16, tag="tri")
            nc.gpsimd.memset(tri[:], 1.0)
            for st in range(4):
                nc.gpsimd.affine_select(
                    out=tri[:, st * BS:(st + 1) * BS],
                    in_=tri[:, st * BS:(st + 1) * BS],
                    compare_op=mybir.AluOpType.is_ge,
                    fill=0.0,
                    base=-P * st,
                    pattern=[[1, BS]],
                    channel_multiplier=-1,
                )
            mA = cp.tile([P, 4 * BS], f16, tag="mA")
            nc.vector.tensor_scalar_max(mA[:], tri[:], parT[:])
            mB = cp.tile([P, 4 * BS], f16, tag="mB")
            nc.vector.tensor_scalar_mul(mB[:], tri[:], parT[:])

            qT = cp.tile([P, 16 * P], f16, tag="qT")    # [d, 2048] own q
            kTm = cp.tile([P, 16 * P], f16, tag="kTm")  # own kT [d, t]
            vstage = cp.tile([P, 16 * P], f16, tag="vstage")  # own v [s, d]
            kT = cp.tile([P, 32 * P], f16, tag="kT")    # parity-ordered [d, 4096]
            vsb = cp.tile([P, 32 * P], f16, tag="vsb")  # parity-ordered v s-tiles
            acc = cp.tile([P, 4 * 4 * BS], f16, tag="acc")
            for j in range(4):
                nc.vector.memset(acc[:, j * 4 * BS:(j + 1) * 4 * BS], 0.0)
            accM = cp.tile([P, 4 * BS], f16, tag="accM")
            o2sb = cp.tile([P, 4 * BS], f16, tag="o2sb")

            # ---- phase 1: projection in [d, t] + RoPE + exchange ----
            H = 64

            def rope(pp, col0, dst_slice, tb):
                ts = slice(tb * BS, (tb + 1) * BS)
                src = pp[:, col0:col0 + BS]
                nc.vector.tensor_mul(dst_slice, src, cst[:, ts])
                tmp = rp.tile([P, BS], f16, tag="ropetmp")
                nc.vector.tensor_mul(tmp[0:H, :], pp[H:P, col0:col0 + BS],
                                     snt[0:H, ts])
                nc.vector.tensor_mul(tmp[H:P, :], pp[0:H, col0:col0 + BS],
                                     snt[H:P, ts])
                nc.vector.tensor_add(dst_slice, dst_slice, tmp[:])

            def issue_exchange(tg):
                nc.scalar.dma_start(cins[tg][:, 0:BS],
                                    kTm[:, tg * BS:(tg + 1) * BS])
                nc.scalar.dma_start(cins[tg][:, BS:2 * BS],
                                    vstage[:, tg * BS:(tg + 1) * BS])
                nc.gpsimd.collective_compute(
                    "AllGather", mybir.AluOpType.bypass,
                    replica_groups=[[0, 1], [2, 3], [4, 5], [6, 7]],
                    ins=[cins[tg][:]], outs=[couts[tg][:]],
                )
                kTv = kT[:].rearrange("p (r x) -> p r x", r=2)
                vsv = vsb[:].rearrange("p (r x) -> p r x", r=2)
                cv = couts[tg][:].rearrange("(r p) x -> p r x", r=2)
                ts = slice(tg * BS, (tg + 1) * BS)
                # sync queue: a collective-gated descriptor here only sits
                # ahead of other ingests/outputs, never phase-1 copies
                nc.sync.dma_start(kTv[:, :, ts], cv[:, :, 0:BS])
                nc.sync.dma_start(vsv[:, :, ts], cv[:, :, BS:2 * BS])

            PCOL = {0: KO, 1: VO, 2: QO}

            def mm(pp, xb, ci, proj):
                nc.tensor.matmul(
                    pp[:, PCOL[proj]:PCOL[proj] + BS],
                    wt[:, (ci * 3 + proj) * P:(ci * 3 + proj + 1) * P],
                    xb[:, ci * BS:(ci + 1) * BS],
                    start=(ci == 0), stop=(ci == 15))

            with (
                tc.tile_pool(name="pps", bufs=2, space="PSUM") as pps,
                tc.tile_pool(name="tps", bufs=2, space="PSUM") as tps,
            ):
                # PE warmup during the DMA-wait head: dummy matmuls into the
                # first pp tile push HAM to K=8/8 before real work arrives
                # (block 0's start=True clears the bank, so garbage is fine)
                pp0 = pps.tile([P, 3 * BS], f32, tag="pp")
                for _ in range(28):
                    nc.tensor.matmul(pp0[:, 0:P], ident[:], ident[:],
                                     start=True, stop=True)
                # preload the exp table while DMA streams in
                dum = rp.tile([P, 16], f16, tag="dum")
                nc.scalar.activation(dum[:], ident[:, 0:16], EXP, scale=1.0)

                for tg in range(4):
                    xb = xbig[tg]
                    pp = pp0 if tg == 0 else pps.tile([P, 3 * BS], f32,
                                                      tag="pp")
                    if tg < 2:
                        # ci-outer: x consumed progressively (DMA-friendly)
                        for ci in range(16):
                            for proj in range(3):
                                mm(pp, xb, ci, proj)
                    else:
                        # proj-outer; block 3 does q first so the phase-1
                        # tail never waits on RoPE-q
                        order = (2, 0, 1) if tg == 3 else (0, 1, 2)
                        for proj in order:
                            for ci in range(16):
                                mm(pp, xb, ci, proj)
                            with tc.high_priority():
                                if proj == 0:
                                    rope(pp, KO,
                                         kTm[:, tg * BS:(tg + 1) * BS], tg)
                                elif proj == 2:
                                    rope(pp, QO,
                                         qT[:, tg * BS:(tg + 1) * BS], tg)
                                else:
                                    vtmp = vtp.tile([P, BS], f16, tag="vtmp")
                                    nc.scalar.copy(vtmp[:], pp[:, VO:VO + BS])
                    with tc.high_priority():
                        if tg < 2:
                            rope(pp, KO, kTm[:, tg * BS:(tg + 1) * BS], tg)
                            vtmp = vtp.tile([P, BS], f16, tag="vtmp")
                            nc.scalar.copy(vtmp[:], pp[:, VO:VO + BS])
                            rope(pp, QO, qT[:, tg * BS:(tg + 1) * BS], tg)
                        # v transposes ride the PE queue after the block MMs
                        for cidx in range(4):
                            tp = tps.tile([P, P], f16, tag="tp")
                            nc.tensor.transpose(
                                tp[:], vtmp[:, cidx * P:(cidx + 1) * P],
                                ident[:])
                            nc.scalar.copy(
                                vstage[:, (4 * tg + cidx) * P:
                                       (4 * tg + cidx + 1) * P], tp[:])
                        issue_exchange(tg)

            # ---- phase 2: attention on [128, 1024] double-tiles ----
            with (
                tc.tile_pool(name="sps", bufs=3, space="PSUM") as sps,
                tc.tile_pool(name="o2ps", bufs=2, space="PSUM") as o2ps,
            ):
                o2s = {}

                plan = []
                for j in range(4):
                    passes = ([(s, None) for s in range(j)]
                              + [(4 + s, None) for s in range(j)]
                              + [(j, mA), (4 + j, mB)])
                    npass = len(passes)
                    for pi, (si, mask) in enumerate(passes):
                        for d in range(2):
                            plan.append((j, si, d, mask,
                                         pi == 0 and d == 0,
                                         pi == npass - 1 and d == 1))

                def emit_scores(item):
                    j, si, d, mask, first, last = item
                    Sd = sps.tile([P, 2 * BS], f32, tag="S")
                    for k in range(2):
                        scol = si * BS + (2 * d + k) * P
                        nc.tensor.matmul(Sd[:, k * BS:(k + 1) * BS],
                                         kT[:, scol:scol + P],
                                         qT[:, j * BS:(j + 1) * BS],
                                         start=True, stop=True)
                    Pt = ptp.tile([P, 2 * BS], f16, tag="Pt")
                    nc.scalar.activation(Pt[:], Sd[:], EXP, scale=SCALE)
                    if mask is not None:
                        # gpsimd: keeps the DVE free for RoPE/acc work
                        nc.gpsimd.tensor_mul(
                            Pt[:], Pt[:], mask[:, d * 2 * BS:(d + 1) * 2 * BS])
                    return (j, si, d, Pt, first, last)

                def emit_av(st8):
                    j, si, d, Pt, first, last = st8
                    if first:
                        o2t = o2ps.tile([P, BS], f32, tag="o2")
                        o2s[j] = o2t
                    for k in range(2):
                        scol = si * BS + (2 * d + k) * P
                        nc.tensor.matmul(o2s[j][:], vsb[:, scol:scol + P],
                                         Pt[:, k * BS:(k + 1) * BS],
                                         start=(first and k == 0),
                                         stop=(last and k == 1))
                    ja = j * 4 * BS + d * 2 * BS
                    nc.vector.tensor_add(acc[:, ja:ja + 2 * BS],
                                         acc[:, ja:ja + 2 * BS], Pt[:])
                    if last:
                        jsl = slice(j * BS, (j + 1) * BS)
                        nc.vector.tensor_scalar_mul(o2sb[:, jsl], o2s[j][:],
                                                    1.0)
                        nc.sync.dma_start(o2out[:, jsl], o2sb[:, jsl])
                        # fold denominator partials: (d0+d1), then (k0+k1)
                        ja4 = j * 4 * BS
                        mrg = rp.tile([P, 2 * BS], f16, tag="mrg")
                        nc.vector.tensor_add(
                            mrg[:], acc[:, ja4:ja4 + 2 * BS],
                            acc[:, ja4 + 2 * BS:ja4 + 4 * BS])
                        nc.vector.tensor_add(
                            accM[:, jsl], mrg[:, 0:BS], mrg[:, BS:2 * BS])
                        nc.sync.dma_start(accout[:, jsl], accM[:, jsl])

                inflight = []
                for item in plan:
                    inflight.append(emit_scores(item))
                    if len(inflight) > 2:
                        emit_av(inflight.pop(0))
                while inflight:
                    emit_av(inflight.pop(0))

    bass_rust.generate_event_semaphores(nc)
    return nc


_CACHE = {}


def _get_nc():
    if "nc" not in _CACHE:
        _CACHE["nc"] = build()
    return _CACHE["nc"]


def _prep_inputs(x, Wq, Wk, Wv, cos, sin):
    perm = np.concatenate([np.arange(0, HD, 2), np.arange(1, HD, 2)])
    wq = Wq[perm].astype(np.float32)
    wk = Wk[perm].astype(np.float32)
    wv = Wv.astype(np.float32)
    # w2[p, (ci, proj, d)] with proj order k|v|q:
    # chunk(ci, proj)[p, d] = Wproj[d, ci*128 + p]
    w3 = np.stack([w.T.reshape(16, P, P) for w in (wk, wv, wq)],
                  axis=1)  # [ci, proj, p, d]
    w2 = np.ascontiguousarray(
        w3.transpose(2, 0, 1, 3).reshape(P, 3 * 16 * P)).astype(np.float16)
    in_maps, orders = [], []
    for c in range(8):
        b, pr = c // 2, c % 2
        order = [pr, pr + 2, pr + 4, pr + 6]
        orders.append(order)
        xb = np.asarray(x[b], np.float32)
        xtp = np.empty((C, T // 2), np.float16)
        cs2 = np.empty((P, T // 2), np.float16)
        sn2 = np.empty((P, T // 2), np.float16)
        for sl, ab in enumerate(order):
            dst = slice(sl * BS, (sl + 1) * BS)
            src = slice(ab * BS, (ab + 1) * BS)
            xtp[:, dst] = xb[src].T
            cblk = cos[src].T.astype(np.float16)  # [64, 512]
            sblk = sin[src].T.astype(np.float16)
            cs2[0:64, dst] = cblk
            cs2[64:P, dst] = cblk
            sn2[0:64, dst] = -sblk
            sn2[64:P, dst] = sblk
        # swizzle x to [p, tg, ci, t]
        xh = np.ascontiguousarray(
            xtp.reshape(16, P, 4, BS).transpose(1, 2, 0, 3).reshape(P, -1))
        parv = np.full((P, 1), float(pr), np.float32)
        in_maps.append({"xt": xh, "w2": w2, "csb": cs2, "snb": sn2,
                        "par": parv})
    return in_maps, orders


def _run(x, Wq, Wk, Wv, cos, sin, trace=False):
    from concourse.bass_utils import run_bass_kernel_spmd
    nc = _get_nc()
    in_maps, orders = _prep_inputs(x, Wq, Wk, Wv, cos, sin)
    res = run_bass_kernel_spmd(nc, in_maps, list(range(8)), trace=trace)
    full = np.empty((B, T, HD), np.float32)
    for c in range(8):
        b, order = c // 2, orders[c]
        o = res.results[c]["o2out"].astype(np.float32).reshape(P, 4, BS)
        den = (res.results[c]["accout"].astype(np.float32)
               .reshape(P, 4, BS).sum(axis=0))
        for j in range(4):
            ab = order[j]
            full[b, ab * BS:(ab + 1) * BS] = (o[:, j, :] / den[j][None, :]).T
    return full, res


def kernel(x, Wq, Wk, Wv, cos, sin):
    return _run(x, Wq, Wk, Wv, cos, sin, trace=False)[0]


# revision 11
# speedup vs baseline: 1.2424x; 1.2424x over previous
"""Single-head causal attention with RoPE on 8 TRN2 NeuronCores (v5).

Sharding: core c -> batch c//2, parity p = c%2 owns the interleaved
512-row q-blocks {p, p+2, p+4, p+6} of T=4096. Each core projects
q/k/v + RoPE only for its OWN 2048 rows; pairs exchange post-RoPE kT
and V via chunked fp16 AllGathers into a parity-ordered kT layout
(parity-0 blocks = kT slots 0-3, parity-1 = slots 4-7), identical
SPMD program on all cores.

v5 changes vs v4:
- q/k projections computed directly in [d, t] layout (weights
  stationary, x streamed) -> RoPE runs on [d, t] tiles with
  partition-offset DVE ops; NO PE transposes or ACT copies for q/k.
- v still projected [d, t] then PE-transposed (4 tiles/block) into
  [s, d] for the exchange.
- Head: DMA emission in exact consumption order, fine-grained first
  chunks, causal masks generated on-device (affine_select + parity
  input) instead of 2MB host DMA.
- Tail: denominator partials merged on-device (2 DVE adds per q-slot)
  -> accout shrinks 2MB -> 0.5MB.
- cin/ingest DMAs ride the scalar HWDGE queue; outputs ride sync, so
  collective-dependent descriptors never block output descriptors.
"""
import numpy as np

B, T, C, HD = 4, 4096, 2048, 128
P = 128
BS = 512
SCALE = float(C) ** -0.5


def build():
    import concourse.bass as bass
    import concourse.mybir as mybir
    import bass_rust
    from concourse.tile import TileContext
    from concourse.masks import make_identity

    f32 = mybir.dt.float32
    f16 = mybir.dt.float16
    EXP = mybir.ActivationFunctionType.Exp

    nc = bass.Bass(num_devices=8)
    # host layouts (see _prep_inputs):
    # xt[p, tg, ci, t] flat; w2[p, (proj k|v|q, ci, d)] flat;
    # csb/snb[d-row, (tb, t)]; par = parity scalar per partition
    xt = nc.declare_dram_parameter("xt", [P, 4 * 16 * BS], f16, isOutput=False)
    w2 = nc.declare_dram_parameter("w2", [P, 3 * 16 * P], f16, isOutput=False)
    csb = nc.declare_dram_parameter("csb", [P, 4 * BS], f16, isOutput=False)
    snb = nc.declare_dram_parameter("snb", [P, 4 * BS], f16, isOutput=False)
    par = nc.declare_dram_parameter("par", [P, 1], f32, isOutput=False)
    # unnormalized AV output [d, j*512+q]; merged denominator partials
    # [s_pos, j*512+q] (host sums the 128 partitions and divides)
    o2out = nc.declare_dram_parameter("o2out", [P, 4 * BS], f16, isOutput=True)
    accout = nc.declare_dram_parameter("accout", [P, 4 * BS], f16,
                                       isOutput=True)

    cins = [nc.dram_tensor(f"cin{t}", [P, 2 * BS], f16, kind="Internal")
            for t in range(4)]
    couts = [nc.dram_tensor(f"cout{t}", [2 * P, 2 * BS], f16, kind="Internal")
             for t in range(4)]

    KO, VO, QO = 0, BS, 2 * BS   # pp column offsets (bank-aligned)

    with TileContext(nc) as tc:
        with (
            tc.tile_pool(name="const", bufs=1) as cp,
            tc.tile_pool(name="xp", bufs=1) as xp,
            tc.tile_pool(name="rot", bufs=3) as rp,
            tc.tile_pool(name="vt", bufs=2) as vtp,
            tc.tile_pool(name="pt", bufs=4) as ptp,
        ):
            # ---- input DMAs interleaved in consumption order (sync) ----
            # w2 layout is ci-major: chunk (ci, proj) at col (ci*3+proj)*128
            wt = cp.tile([P, 3 * 16 * P], f16, tag="wt")
            xbig = [None] * 4

            def load_x(tg, interleave_w=False):
                xb = xp.tile([P, 16 * BS], f16, tag=f"xbig{tg}")
                for d in range(4):
                    if interleave_w:
                        ws = slice(d * 12 * P, (d + 1) * 12 * P)
                        nc.sync.dma_start(wt[:, ws], w2[:, ws])
                    base = tg * 16 * BS + d * 4 * BS
                    nc.sync.dma_start(
                        xb[:, d * 4 * BS:(d + 1) * 4 * BS],
                        xt[:, base:base + 4 * BS])
                xbig[tg] = xb

            load_x(0, interleave_w=True)
            cst = cp.tile([P, 4 * BS], f16, tag="cst")
            snt = cp.tile([P, 4 * BS], f16, tag="snt")
            nc.sync.dma_start(cst[:, 0:BS], csb[:, 0:BS])
            nc.sync.dma_start(snt[:, 0:BS], snb[:, 0:BS])
            parT = cp.tile([P, 1], f32, tag="parT")
            nc.sync.dma_start(parT[:], par[:])
            load_x(1)
            nc.sync.dma_start(cst[:, BS:4 * BS], csb[:, BS:4 * BS])
            nc.sync.dma_start(snt[:, BS:4 * BS], snb[:, BS:4 * BS])
            load_x(2)
            load_x(3)

            ident = cp.tile([P, P], f16, tag="ident")
            make_identity(nc, ident[:])

            # on-device causal masks: tri[p, st*512+q] = (p + 128*st <= q)
            tri = cp.tile([P, 4 * BS], f16, tag="tri")
            nc.gpsimd.memset(tri[:], 1.0)
            for st in range(4):
                nc.gpsimd.affine_select(
                    out=tri[:, st * BS:(st + 1) * BS],
                    in_=tri[:, st * BS:(st + 1) * BS],
                    compare_op=mybir.AluOpType.is_ge,
                    fill=0.0,
                    base=-P * st,
                    pattern=[[1, BS]],
                    channel_multiplier=-1,
                )
            mA = cp.tile([P, 4 * BS], f16, tag="mA")
            nc.vector.tensor_scalar_max(mA[:], tri[:], parT[:])
            mB = cp.tile([P, 4 * BS], f16, tag="mB")
            nc.vector.tensor_scalar_mul(mB[:], tri[:], parT[:])

            qT = cp.tile([P, 16 * P], f16, tag="qT")    # [d, 2048] own q
            kTm = cp.tile([P, 16 * P], f16, tag="kTm")  # own kT [d, t]
            vstage = cp.tile([P, 16 * P], f16, tag="vstage")  # own v [s, d]
            kT = cp.tile([P, 32 * P], f16, tag="kT")    # parity-ordered [d, 4096]
            vsb = cp.tile([P, 32 * P], f16, tag="vsb")  # parity-ordered v s-tiles
            acc = cp.tile([P, 4 * 4 * BS], f16, tag="acc")
            for j in range(4):
                nc.vector.memset(acc[:, j * 4 * BS:(j + 1) * 4 * BS], 0.0)
            accM = cp.tile([P, 4 * BS], f16, tag="accM")
            o2sb = cp.tile([P, 4 * BS], f16, tag="o2sb")

            # ---- phase 1: projection in [d, t] + RoPE + exchange ----
            H = 64

            def rope(pp, col0, dst_slice, tb):
                ts = slice(tb * BS, (tb + 1) * BS)
                src = pp[:, col0:col0 + BS]
                nc.vector.tensor_mul(dst_slice, src, cst[:, ts])
                tmp = rp.tile([P, BS], f16, tag="ropetmp")
                nc.vector.tensor_mul(tmp[0:H, :], pp[H:P, col0:col0 + BS],
                                     snt[0:H, ts])
                nc.vector.tensor_mul(tmp[H:P, :], pp[0:H, col0:col0 + BS],
                                     snt[H:P, ts])
                nc.vector.tensor_add(dst_slice, dst_slice, tmp[:])

            def issue_exchange(tg):
                nc.scalar.dma_start(cins[tg][:, 0:BS],
                                    kTm[:, tg * BS:(tg + 1) * BS])
                nc.scalar.dma_start(cins[tg][:, BS:2 * BS],
                                    vstage[:, tg * BS:(tg + 1) * BS])
                nc.gpsimd.collective_compute(
                    "AllGather", mybir.AluOpType.bypass,
                    replica_groups=[[0, 1], [2, 3], [4, 5], [6, 7]],
                    ins=[cins[tg][:]], outs=[couts[tg][:]],
                )
                kTv = kT[:].rearrange("p (r x) -> p r x", r=2)
                vsv = vsb[:].rearrange("p (r x) -> p r x", r=2)
                cv = couts[tg][:].rearrange("(r p) x -> p r x", r=2)
                ts = slice(tg * BS, (tg + 1) * BS)
                # sync queue: a collective-gated descriptor here only sits
                # ahead of other ingests/outputs, never phase-1 copies
                nc.sync.dma_start(kTv[:, :, ts], cv[:, :, 0:BS])
                nc.sync.dma_start(vsv[:, :, ts], cv[:, :, BS:2 * BS])

            PCOL = {0: KO, 1: VO, 2: QO}

            def mm(pp, xb, ci, proj):
                nc.tensor.matmul(
                    pp[:, PCOL[proj]:PCOL[proj] + BS],
                    wt[:, (ci * 3 + proj) * P:(ci * 3 + proj + 1) * P],
                    xb[:, ci * BS:(ci + 1) * BS],
                    start=(ci == 0), stop=(ci == 15))

            with (
                tc.tile_pool(name="pps", bufs=2, space="PSUM") as pps,
                tc.tile_pool(name="tps", bufs=2, space="PSUM") as tps,
            ):
                # PE warmup during the DMA-wait head: dummy matmuls into the
                # first pp tile push HAM to K=8/8 before real work arrives
                # (block 0's start=True clears the bank, so garbage is fine)
                pp0 = pps.tile([P, 3 * BS], f32, tag="pp")
                for _ in range(28):
                    nc.tensor.matmul(pp0[:, 0:P], ident[:], ident[:],
                                     start=True, stop=True)
                # preload the exp table while DMA streams in
                dum = rp.tile([P, 16], f16, tag="dum")
                nc.scalar.activation(dum[:], ident[:, 0:16], EXP, scale=1.0)

                for tg in range(4):
                    xb = xbig[tg]
                    pp = pp0 if tg == 0 else pps.tile([P, 3 * BS], f32,
                                                      tag="pp")
                    if tg < 2:
                        # ci-outer: x consumed progressively (DMA-friendly)
                        for ci in range(16):
                            for proj in range(3):
                                mm(pp, xb, ci, proj)
                    else:
                        # proj-outer; block 3 does q first so the phase-1
                        # tail never waits on RoPE-q
                        order = (2, 0, 1) if tg == 3 else (0, 1, 2)
                        for proj in order:
                            for ci in range(16):
                                mm(pp, xb, ci, proj)
                            with tc.high_priority():
                                if proj == 0:
                                    rope(pp, KO,
                                         kTm[:, tg * BS:(tg + 1) * BS], tg)
                                elif proj == 2:
                                    rope(pp, QO,
                                         qT[:, tg * BS:(tg + 1) * BS], tg)
                                else:
                                    vtmp = vtp.tile([P, BS], f16, tag="vtmp")
                                    nc.scalar.copy(vtmp[:], pp[:, VO:VO + BS])
                    with tc.high_priority():
                        if tg < 2:
                            rope(pp, KO, kTm[:, tg * BS:(tg + 1) * BS], tg)
                            vtmp = vtp.tile([P, BS], f16, tag="vtmp")
                            nc.scalar.copy(vtmp[:], pp[:, VO:VO + BS])
                            rope(pp, QO, qT[:, tg * BS:(tg + 1) * BS], tg)
                        # v transposes ride the PE queue after the block MMs
                        for cidx in range(4):
                            tp = tps.tile([P, P], f16, tag="tp")
                            nc.tensor.transpose(
                                tp[:], vtmp[:, cidx * P:(cidx + 1) * P],
                                ident[:])
                            nc.scalar.copy(
                                vstage[:, (4 * tg + cidx) * P:
                                       (4 * tg + cidx + 1) * P], tp[:])
                    # NOT high-priority: the collective-gated ingest
                    # descriptors must sit behind the x-stream on sync
                    issue_exchange(tg)

            # ---- phase 2: attention on [128, 1024] double-tiles ----
            with (
                tc.tile_pool(name="sps", bufs=3, space="PSUM") as sps,
                tc.tile_pool(name="o2ps", bufs=2, space="PSUM") as o2ps,
            ):
                o2s = {}

                plan = []
                for j in range(4):
                    passes = ([(s, None) for s in range(j)]
                              + [(4 + s, None) for s in range(j)]
                              + [(j, mA), (4 + j, mB)])
                    npass = len(passes)
                    for pi, (si, mask) in enumerate(passes):
                        for d in range(2):
                            plan.append((j, si, d, mask,
                                         pi == 0 and d == 0,
                                         pi == npass - 1 and d == 1))

                def emit_scores(item):
                    j, si, d, mask, first, last = item
                    Sd = sps.tile([P, 2 * BS], f32, tag="S")
                    for k in range(2):
                        scol = si * BS + (2 * d + k) * P
                        nc.tensor.matmul(Sd[:, k * BS:(k + 1) * BS],
                                         kT[:, scol:scol + P],
                                         qT[:, j * BS:(j + 1) * BS],
                                         start=True, stop=True)
                    Pt = ptp.tile([P, 2 * BS], f16, tag="Pt")
                    nc.scalar.activation(Pt[:], Sd[:], EXP, scale=SCALE)
                    if mask is not None:
                        nc.vector.tensor_mul(
                            Pt[:], Pt[:], mask[:, d * 2 * BS:(d + 1) * 2 * BS])
                    return (j, si, d, Pt, first, last)

                def emit_av(st8):
                    j, si, d, Pt, first, last = st8
                    if first:
                        o2t = o2ps.tile([P, BS], f32, tag="o2")
                        o2s[j] = o2t
                    for k in range(2):
                        scol = si * BS + (2 * d + k) * P
                        nc.tensor.matmul(o2s[j][:], vsb[:, scol:scol + P],
                                         Pt[:, k * BS:(k + 1) * BS],
                                         start=(first and k == 0),
                                         stop=(last and k == 1))
                    ja = j * 4 * BS + d * 2 * BS
                    nc.vector.tensor_add(acc[:, ja:ja + 2 * BS],
                                         acc[:, ja:ja + 2 * BS], Pt[:])
                    if last:
                        jsl = slice(j * BS, (j + 1) * BS)
                        nc.vector.tensor_scalar_mul(o2sb[:, jsl], o2s[j][:],
                                                    1.0)
                        nc.sync.dma_start(o2out[:, jsl], o2sb[:, jsl])
                        # fold denominator partials: (d0+d1), then (k0+k1)
                        ja4 = j * 4 * BS
                        mrg = rp.tile([P, 2 * BS], f16, tag="mrg")
                        nc.vector.tensor_add(
                            mrg[:], acc[:, ja4:ja4 + 2 * BS],
                            acc[:, ja4 + 2 * BS:ja4 + 4 * BS])
                        nc.vector.tensor_add(
                            accM[:, jsl], mrg[:, 0:BS], mrg[:, BS:2 * BS])
                        nc.sync.dma_start(accout[:, jsl], accM[:, jsl])

                inflight = []
                for item in plan:
                    inflight.append(emit_scores(item))
                    if len(inflight) > 2:
                        emit_av(inflight.pop(0))
                while inflight:
                    emit_av(inflight.pop(0))

    bass_rust.generate_event_semaphores(nc)
    return nc


_CACHE = {}


def _get_nc():
    if "nc" not in _CACHE:
        _CACHE["nc"] = build()
    return _CACHE["nc"]


def _prep_inputs(x, Wq, Wk, Wv, cos, sin):
    perm = np.concatenate([np.arange(0, HD, 2), np.arange(1, HD, 2)])
    wq = Wq[perm].astype(np.float32)
    wk = Wk[perm].astype(np.float32)
    wv = Wv.astype(np.float32)
    # w2[p, (ci, proj, d)] with proj order k|v|q:
    # chunk(ci, proj)[p, d] = Wproj[d, ci*128 + p]
    w3 = np.stack([w.T.reshape(16, P, P) for w in (wk, wv, wq)],
                  axis=1)  # [ci, proj, p, d]
    w2 = np.ascontiguousarray(
        w3.transpose(2, 0, 1, 3).reshape(P, 3 * 16 * P)).astype(np.float16)
    in_maps, orders = [], []
    for c in range(8):
        b, pr = c // 2, c % 2
        order = [pr, pr + 2, pr + 4, pr + 6]
        orders.append(order)
        xb = np.asarray(x[b], np.float32)
        xtp = np.empty((C, T // 2), np.float16)
        cs2 = np.empty((P, T // 2), np.float16)
        sn2 = np.empty((P, T // 2), np.float16)
        for sl, ab in enumerate(order):
            dst = slice(sl * BS, (sl + 1) * BS)
            src = slice(ab * BS, (ab + 1) * BS)
            xtp[:, dst] = xb[src].T
            cblk = cos[src].T.astype(np.float16)  # [64, 512]
            sblk = sin[src].T.astype(np.float16)
            cs2[0:64, dst] = cblk
            cs2[64:P, dst] = cblk
            sn2[0:64, dst] = -sblk
            sn2[64:P, dst] = sblk
        # swizzle x to [p, tg, ci, t]
        xh = np.ascontiguousarray(
            xtp.reshape(16, P, 4, BS).transpose(1, 2, 0, 3).reshape(P, -1))
        parv = np.full((P, 1), float(pr), np.float32)
        in_maps.append({"xt": xh, "w2": w2, "csb": cs2, "snb": sn2,
                        "par": parv})
    return in_maps, orders


def _run(x, Wq, Wk, Wv, cos, sin, trace=False):
    from concourse.bass_utils import run_bass_kernel_spmd
    nc = _get_nc()
    in_maps, orders = _prep_inputs(x, Wq, Wk, Wv, cos, sin)
    res = run_bass_kernel_spmd(nc, in_maps, list(range(8)), trace=trace)
    full = np.empty((B, T, HD), np.float32)
    for c in range(8):
        b, order = c // 2, orders[c]
        o = res.results[c]["o2out"].astype(np.float32).reshape(P, 4, BS)
        den = (res.results[c]["accout"].astype(np.float32)
               .reshape(P, 4, BS).sum(axis=0))
        for j in range(4):
            ab = order[j]
            full[b, ab * BS:(ab + 1) * BS] = (o[:, j, :] / den[j][None, :]).T
    return full, res


def kernel(x, Wq, Wk, Wv, cos, sin):
    return _run(x, Wq, Wk, Wv, cos, sin, trace=False)[0]
